# revision 71
# baseline (speedup 1.0000x reference)
# Multi-head attention (B=2, S=2048, D=1024, H=16, head_dim=64) with bool mask,
# sharded across 8 TRN2 NeuronCores: core c -> batch c//4, heads 4*(c%4)..4*(c%4)+3.
#
# Per-core device kernel (scores computed transposed: scoresT[k, q]):
#   scoresT = K @ Q^T                 (PE bf16, lhsT = K^T strip, rhs = Q^T)
#   atp     = exp(scoresT/8)          (ACT exp scale=1/8, psum -> psum bf16)
#   at      = atp * (1-m)T            (DVE mult, psum -> SBUF bf16)
#   out[q,d] += at_chunk^T @ [V|1]    (PE bf16: lhsT = at chunk (stationary),
#                                      rhs = V'[128,65]; col 64 accumulates Z)
#   out     = psO[:, :, 0:64] / Z     (DVE reciprocal + broadcast multiply)
#
# The AV matmul uses the attention chunk as the stationary operand so the
# output lands non-transposed ([q, d] with q on partitions): free size is 65
# instead of 512 per instruction (half the PE cycles of the V-stationary
# form) and the final PE transposes disappear entirely.
#
# Host side (inside kernel()): slice per-core shards, pre-transpose Q/K per
# head ([64, S] head-dim-major, bf16), pre-transpose the inverted mask to
# bf16, reassemble the 8 per-core bf16 outputs into the full f32 output.

import sys

import numpy as np

for _p in ("/opt/trn_rl_repo",):
    if _p not in sys.path:
        sys.path.insert(0, _p)

import ml_dtypes

import concourse.bass as bass  # noqa: F401  (engine types reachable via nc)
import concourse.tile as tile
from concourse import bacc, mybir
from concourse.bass_utils import run_bass_kernel_spmd
from concourse.masks import make_identity

F32 = mybir.dt.float32
BF16 = mybir.dt.bfloat16
FP8 = mybir.dt.float8e4

S = 2048          # sequence length
HD = 64           # head dim
HPC = 4           # heads per core
NCORES = 8
B = 2
H = 16
D = H * HD

# Work-assignment schedule (per k-strip within a group), tuned against the
# cost model: which strips' exp runs as the DVE bit trick, which masks are
# folded into the QK psum accumulation on PE, which multiply on Pool.
SCHED = {
    "exp_dve": (1, 3, 5, 7, 9, 11, 13),
    "fold": (0, 1, 3, 4, 5, 7, 9, 10, 11, -2, -1),
    "pool": (6, 8),
    "lag": 7,
    "warmup": 24,
}


def build_program(s=S, reps=1):
    """Build the single-core SPMD program. Returns the compiled Bacc object.

    reps>1 emits the whole body (loads+compute+stores) that many times in one
    NEFF — used to measure device time by wall-clock differencing."""
    nc = bacc.Bacc()

    KS = s // 128            # number of k strips
    QG = 1024 if s >= 1024 else s   # q group width (ACT/DVE instruction width)
    NQG = s // QG            # q groups
    NQC = max(QG // 512, 1)  # 512-wide matmul chunks per q group (psum bank)
    QC = min(512, QG)        # matmul chunk width
    NCH = QG // 128          # 128-wide q chunks per group (AV granularity)
    CPB = 4                  # psO chunks per 2KB psum bank (zero region)
    NB = (NCH + CPB - 1) // CPB  # psO sub-tiles (1 bank each)
    LAG = min(4, KS)         # AV strips emitted this many strips behind QK

    qkT_d = nc.declare_dram_parameter("qkT", [2, HPC * HD, s], BF16, isOutput=False)
    v_d = nc.declare_dram_parameter("v", [s, HPC * HD], BF16, isOutput=False)
    nmT_d = nc.declare_dram_parameter("nmT", [s, s], BF16, isOutput=False)
    nmT8_d = nc.declare_dram_parameter("nmT8", [s, s], FP8, isOutput=False)
    out_d = nc.declare_dram_parameter("out", [s, HPC * HD], BF16, isOutput=True)

    # DRAM views with the k/q axis split into strips of 128 partitions
    nm_view = nmT_d[:].rearrange("(ks p) q -> p ks q", p=128)
    nm8_view = nmT8_d[:].rearrange("(ks p) q -> p ks q", p=128)
    v_view = v_d[:].rearrange("(ks p) (h d) -> p ks h d", p=128, h=HPC)
    out_view = out_d[:].rearrange("(sq p) c -> p sq c", p=128)

    with tile.TileContext(nc) as tc:
        with (
            tc.tile_pool(name="const", bufs=1) as const,
            tc.tile_pool(name="wq", bufs=1) as wq,
            tc.tile_pool(name="attn", bufs=20) as apool,
            tc.tile_pool(name="xsb", bufs=3) as xpool,
            tc.tile_pool(name="stat", bufs=4) as spool,
            tc.tile_pool(name="oasm", bufs=1) as opool,
            tc.tile_pool(name="psS", bufs=3, space="PSUM") as psS_pool,
            tc.tile_pool(name="psOa", bufs=1, space="PSUM") as psOa_pool,
            tc.tile_pool(name="psOb", bufs=1, space="PSUM") as psOb_pool,
        ):
            # Preload the exp table (emitted before any real exp; runs while
            # the first DMAs stream).
            warm = const.tile([128, 1], F32)
            nc.vector.memset(warm, 0.0)
            nc.scalar.activation(warm, warm, mybir.ActivationFunctionType.Exp)

            # Mask folding constants: psS += 224*nm via an fp8 DoubleRow
            # matmul (exact: the mask is 0/1 and 224 is representable in
            # e4m3 whose max is 240), then exp gets bias -28 so masked
            # entries become e^-28 ~ 7e-13 — no elementwise mask op at all.
            # DoubleRow weights: k-tile 0 = 224*I, k-tile 1 = 0 (the second
            # tile's moving data is arbitrary padding).
            identf = const.tile([128, 128], F32)
            make_identity(nc, identf)
            i448p = const.tile([128, 2, 128], FP8)
            nc.vector.memset(i448p, 0.0)
            nc.vector.tensor_scalar_mul(i448p[:, 0, :], identf, 224.0)
            bias56 = const.tile([128, 1], F32)
            nc.vector.memset(bias56, -28.0)

            # Warm the PE HAM clock gate while input DMAs run: ~3us of dummy
            # matmuls so the first real QKs run at 2.4GHz.
            zb = const.tile([128, 128], BF16)
            nc.vector.memset(zb, 0.0)
            for _ in range(SCHED.get("warmup", 24)):
                wmm = psS_pool.tile([128, QG], F32, tag="psS")
                nc.tensor.matmul(
                    wmm[:, :128], lhsT=zb[0:64, :], rhs=zb[0:64, :],
                    start=True, stop=True,
                )

            def qk_src(pair):
                return qkT_d[:, 128 * pair:128 * pair + 128, :].rearrange(
                    "t p s -> p t s"
                )

            def emit_body():
                # Q^T / K^T head pairs: [128, s] (head 2p on partitions 0-63,
                # head 2p+1 on partitions 64-127).
                qks = []
                for pair in range(HPC // 2):
                    qk = wq.tile([128, 2, s], BF16, tag=f"qkT{pair}")
                    qks.append(qk)
                # V' staging: [128, ks, h, 65] with a ones column at 64 so the
                # AV matmul's 65th output column accumulates the softmax
                # denominator Z. V lands via interleaved DMA; the ones column
                # is memset once (disjoint subtile, no dependency on the DMA).
                vps = wq.tile([128, KS, HPC, HD + 1], BF16, tag="vps")
                nm_sb = wq.tile([128, KS, s], BF16, tag="nm")
                # fp8 copy of the mask for the DoubleRow PE folds, flat with
                # a 512B tail pad so the (ignored) second k-tile window of
                # the last chunk stays in range
                nm8 = wq.tile([128, KS * s + 512], FP8, tag="nm8")
                nc.vector.memset(vps[:, :, :, HD:HD + 1], 1.0)
                nc.vector.memset(nm8[:, KS * s:], 0.0)

                # DMA choreography (s=2048): two queues only — SP (nc.sync)
                # and Pool SWDGE (nc.gpsimd) — so the ACT and DVE sequencers
                # are never blocked behind a DMA wait. Ordered by first use:
                # K strips + first Q group first, mask halves interleaved,
                # V early (AV matmuls sit in the in-order PE queue).
                QH = QG  # nm half width
                if s == 2048:
                    A, Bq = nc.sync, nc.gpsimd
                    # The model's DMA device is effectively serial, so the
                    # ordering across the queues is what matters: the
                    # first-QK inputs lead on SP (issued at t=0; the ACT
                    # queue is busy with the exp-table warmup), then mask
                    # halves at the consumption rate, with V and the second
                    # head-pair deferred to their first use.
                    A.dma_start(out=qks[0][:, 1, 0:256], in_=qk_src(0)[:, 1, 0:256])
                    A.dma_start(out=qks[0][:, 0, 0:512], in_=qk_src(0)[:, 0, 0:512])
                    A.dma_start(out=qks[0][:, 0, 512:QG], in_=qk_src(0)[:, 0, 512:QG])
                    A.dma_start(out=qks[0][:, 1, 256:1024], in_=qk_src(0)[:, 1, 256:1024])
                    Bq.dma_start(out=vps[:, :, 0, 0:HD], in_=v_view[:, :, 0])
                    A.dma_start(out=nm_sb[:, 0, 0:QH], in_=nm_view[:, 0, 0:QH])
                    Bq.dma_start(out=nm_sb[:, 1, 0:QH], in_=nm_view[:, 1, 0:QH])
                    A.dma_start(out=nm_sb[:, 2, 0:QH], in_=nm_view[:, 2, 0:QH])
                    A.dma_start(out=qks[0][:, 1, 1024:2048], in_=qk_src(0)[:, 1, 1024:2048])
                    Bq.dma_start(out=nm_sb[:, 3, 0:QH], in_=nm_view[:, 3, 0:QH])
                    for ks in range(4, KS):
                        (A if ks % 2 == 0 else Bq).dma_start(
                            out=nm_sb[:, ks, 0:QH], in_=nm_view[:, ks, 0:QH]
                        )
                        if ks == 8:
                            A.dma_start(out=qks[0][:, 0, QG:2 * QG],
                                        in_=qk_src(0)[:, 0, QG:2 * QG])
                    # second batch: q-group-1 mask halves; then the fp8 mask
                    # copy (PE folds start in group 2, late strips first),
                    # V heads 1-3 and the second head pair (needed only from
                    # groups 2/4/6 at ~33/66/100us).
                    Bq.dma_start(out=vps[:, :, 1, 0:HD], in_=v_view[:, :, 1])
                    for ks in range(KS):
                        (A if ks % 2 == 0 else Bq).dma_start(
                            out=nm_sb[:, ks, QH:2 * QH], in_=nm_view[:, ks, QH:2 * QH]
                        )
                    for ks in range(KS):
                        (A if ks % 2 == 0 else Bq).dma_start(
                            out=nm8[:, ks * s:(ks + 1) * s], in_=nm8_view[:, ks]
                        )
                    A.dma_start(out=qks[1], in_=qk_src(1))
                    Bq.dma_start(out=vps[:, :, 2, 0:HD], in_=v_view[:, :, 2])
                    Bq.dma_start(out=vps[:, :, 3, 0:HD], in_=v_view[:, :, 3])
                else:
                    A, Bq = nc.sync, nc.gpsimd
                    A.dma_start(out=qks[0], in_=qk_src(0))
                    for hh in range(HPC):
                        Bq.dma_start(out=vps[:, :, hh, 0:HD], in_=v_view[:, :, hh])
                    for pair in range(1, HPC // 2):
                        A.dma_start(out=qks[pair], in_=qk_src(pair))
                    for ks in range(KS):
                        (A if ks % 2 == 0 else Bq).dma_start(
                            out=nm_sb[:, ks, :], in_=nm_view[:, ks, :]
                        )
                    for ks in range(KS):
                        (A if ks % 2 == 0 else Bq).dma_start(
                            out=nm8[:, ks * s:(ks + 1) * s], in_=nm8_view[:, ks]
                        )

                out_asm = opool.tile([128, KS, HPC * HD], BF16)

                def _alloc_psO():
                    pools = [psOa_pool, psOb_pool]
                    tiles = []
                    for t in range(NB):
                        psO_t = pools[t].tile(
                            [128, min(CPB, NCH), 128], F32, tag=f"psO{t}"
                        )
                        tiles.append(psO_t)
                    return tiles

                def emit_carry(carry):
                    """Last two AV strips (lag-2 emission) + finalize: Z
                    reciprocal, broadcast normalize, and the output DMA once
                    the last head of a q-group completes."""
                    ch, cqg, cpsO, at_tail = carry
                    for i, (cat, cks) in enumerate(at_tail):
                        last = i == len(at_tail) - 1
                        for j in range(NCH):
                            nc.tensor.matmul(
                                cpsO[j // CPB][:, j % CPB, 0:HD + 1],
                                lhsT=cat[:, j * 128:(j + 1) * 128],
                                rhs=vps[:, cks, ch, :],
                                # start/stop once per psO sub-tile (4 chunks
                                # of 512B fill its 2KB zero region)
                                start=(cks == 0 and j % CPB == 0),
                                stop=last and (j % CPB == CPB - 1 or j == NCH - 1),
                            )
                    final = ch == HPC - 1 and cqg == NQG - 1
                    rec = spool.tile([128, NCH], F32)
                    step = min(max(NCH // 2, 1), CPB)
                    for lo in range(0, NCH, step):
                        hi = min(lo + step, NCH)
                        sq0 = cqg * NCH + lo
                        ct = cpsO[lo // CPB]
                        cl = lo % CPB
                        # per-half reciprocal over its own psO sub-tile: on
                        # the final group the first half's normalize + DMA
                        # then only depends on the first exp/mask half's AVs
                        nc.vector.reciprocal(rec[:, lo:hi], ct[:, cl:cl + hi - lo, HD])
                        nc.vector.tensor_mul(
                            out_asm[:, sq0:sq0 + hi - lo, ch * HD:(ch + 1) * HD],
                            ct[:, cl:cl + hi - lo, 0:HD],
                            rec[:, lo:hi].to_broadcast([128, hi - lo, HD]),
                        )
                        if ch == HPC - 1:
                            # the final group's DMAs both ride SP: the Pool
                            # SWDGE prep (~1.1us) would sit on the critical
                            # path at the very end of the program
                            eng = nc.gpsimd if (lo > 0 and not final) else nc.sync
                            eng.dma_start(
                                out=out_view[:, sq0:sq0 + hi - lo, :],
                                in_=out_asm[:, sq0:sq0 + hi - lo, :],
                            )

                carry = None
                groups = [(h, qg) for h in range(HPC) for qg in range(NQG)]
                NG = len(groups)

                # Work-assignment tables (balanced so each engine sits at
                # ~60-70% duty per group, with slow per-strip paths — Pool
                # masks at ~2.1us, DVE bit-trick exps — spread out so no
                # engine ever falls far enough behind to block the in-order
                # PE queue):
                #   exp:  ACT (exact) | DVE (exp2 bit trick, ~26% of strips)
                #   mask: DVE mult | POOL mult | PE (448*nm folded into QK)
                EXP_DVE_KS = {k for k in SCHED["exp_dve"] if k < max(KS - 2, 2)}
                FOLD_KS = {k % KS if k >= 0 else (KS + k) for k in SCHED["fold"]}
                POOL_KS = {k for k in SCHED["pool"] if k < KS}

                def exp_on_dve(gi, ks):
                    if gi < min(2, NG - 2):
                        return False
                    return ks in EXP_DVE_KS

                def mask_mode(gi, ks):
                    """Head groups stay on DVE: the PE fold needs the mask
                    strip ~2 strips earlier than the multiply, and the head
                    is exactly where the mask DMAs are marginal. Pool masks
                    skip group 2's first half (Pool is still issuing input
                    DMAs) and the final group's tail (a late Pool mask
                    would gate the output chain)."""
                    if gi < min(2, NG - 2):
                        return "DVE"
                    if ks in FOLD_KS:
                        return "PE"
                    if ks in POOL_KS:
                        if gi == 2 and ks < KS // 2:
                            return "DVE"
                        return "POOL"
                    return "DVE"

                for gi, (h, qg) in enumerate(groups):
                    base = 64 * (h % 2)
                    qt_r = qks[h // 2][:, 0, :]
                    kt_r = qks[h // 2][:, 1, :]
                    q0 = qg * QG
                    # AV lag: a late mask multiply (Pool backlog, in-flight
                    # nm DMA at the head) must not block the in-order PE
                    # queue right before the QK the ACT engine is waiting
                    # on. The end-of-group AV backlog overlaps the last
                    # exps (only the final strip's AVs are chain-critical).
                    lag = min(SCHED.get("lag", 4), KS)
                    last_g = gi == len(groups) - 1
                    psO = None
                    ats = {}
                    for ks in range(KS):
                        if ks - lag in ats:
                            at2 = ats.pop(ks - lag)
                            if psO is None:
                                psO = _alloc_psO()
                            for j in range(NCH):
                                nc.tensor.matmul(
                                    psO[j // CPB][:, j % CPB, 0:HD + 1],
                                    lhsT=at2[:, j * 128:(j + 1) * 128],
                                    rhs=vps[:, ks - lag, h, :],
                                    start=(ks == lag and j % CPB == 0),
                                    stop=False,
                                )
                        mmode = mask_mode(gi, ks)
                        psS = psS_pool.tile([128, QG], F32, tag="psS")
                        for qc in range(NQC):
                            qsl = slice(q0 + qc * QC, q0 + (qc + 1) * QC)
                            nc.tensor.matmul(
                                psS[:, qc * QC:(qc + 1) * QC],
                                lhsT=kt_r[base:base + HD, ks * 128:(ks + 1) * 128],
                                rhs=qt_r[base:base + HD, qsl],
                                start=True,
                                stop=(mmode != "PE"),
                            )
                            if mmode == "PE":
                                # fold the mask into the psum accumulation
                                # (psS += 224*nm; exp then gets bias -28) as an
                                # fp8 DoubleRow matmul: exact (mask is 0/1,
                                # 224 is representable) at half the cycles;
                                # k-tile 1 has zero weights so its moving
                                # window is don't-care.
                                off = ks * s + qsl.start
                                rhs8 = nm8[:, off:off + 2 * QC].rearrange(
                                    "p (two f) -> p two f", two=2
                                )
                                nc.tensor.matmul(
                                    psS[:, qc * QC:(qc + 1) * QC],
                                    lhsT=i448p,
                                    rhs=rhs8,
                                    start=False,
                                    stop=True,
                                    perf_mode=mybir.MatmulPerfMode.DoubleRow,
                                )

                        at = apool.tile([128, QG], BF16, tag="at")
                        folded = mmode == "PE"
                        if exp_on_dve(gi, ks):
                            # exp2 bit trick on DVE: y = trunc(x*0.125*
                            # log2(e)*128 + (127*128 - 5.5)) as int16 IS the
                            # bf16 pattern of ~exp(x/8) (softmax
                            # normalization absorbs the approximation's
                            # constant factor; the -5.5 centers its mean so
                            # mixing with exact-exp strips stays unbiased).
                            # With the PE mask fold, the -448 lands in the
                            # exponent field: masked entries become ~2^-69.
                            c1 = 0.125 * 1.4426950408889634 * 128.0
                            c2 = 16250.5 - (224.0 * c1 if folded else 0.0)
                            nc.vector.tensor_scalar(
                                at[:].bitcast(mybir.dt.int16), psS,
                                c1, c2,
                                mybir.AluOpType.mult,
                                mybir.AluOpType.add,
                            )
                        else:
                            # last strip of every group: exp (+mask) in
                            # halves so the carry AV chunks 0-3 start half
                            # an exp earlier (subtile deps)
                            nsp = 2 if (QG >= 1024 and (
                                ks == KS - 1 or (gi == 0 and ks == 0)
                            )) else 1
                            for sp in range(nsp):
                                sl = slice(sp * QG // nsp, (sp + 1) * QG // nsp)
                                nc.scalar.activation(
                                    at[:, sl], psS[:, sl],
                                    mybir.ActivationFunctionType.Exp,
                                    scale=0.125,
                                    bias=bias56[:] if folded else 0.0,
                                )
                                if mmode == "DVE":
                                    nc.vector.tensor_mul(
                                        at[:, sl], at[:, sl],
                                        nm_sb[:, ks, q0 + sl.start:q0 + sl.stop],
                                    )
                        if not folded and (mmode == "POOL" or exp_on_dve(gi, ks)):
                            eng = nc.gpsimd if mmode == "POOL" else nc.vector
                            eng.tensor_mul(at, at, nm_sb[:, ks, q0:q0 + QG])
                        ats[ks] = at
                        # carry (norms on DVE) emitted after strip 1's ops:
                        # at a group boundary the new group's strip-1 DVE
                        # affine must not queue behind the old group's
                        # normalize chain, or the psS rotation stalls ACT
                        if ks == min(1, KS - 1) and carry is not None:
                            emit_carry(carry)
                            carry = None
                    if psO is None:
                        psO = _alloc_psO()
                    tail = sorted(ats.items())
                    carry = (h, qg, psO, [(a, k) for k, a in tail])
                emit_carry(carry)

            for _ in range(reps):
                emit_body()
    nc.compile()
    return nc


_CACHE = {}


def _get_nc():
    if "nc" not in _CACHE:
        _CACHE["nc"] = build_program()
    return _CACHE["nc"]


def make_in_maps(q, k, v, mask, s=S):
    """Shard full inputs into 8 per-core input maps (host-side layout prep)."""
    q = np.asarray(q, dtype=np.float32)
    k = np.asarray(k, dtype=np.float32)
    v = np.asarray(v, dtype=np.float32)
    mask = np.asarray(mask)
    nh = q.shape[-1] // HD
    in_maps = []
    for c in range(NCORES):
        b, g = divmod(c, NCORES // B)
        h0 = HPC * g
        qs = q[b].reshape(s, nh, HD)[:, h0:h0 + HPC, :]      # [s, HPC, 64]
        ks_ = k[b].reshape(s, nh, HD)[:, h0:h0 + HPC, :]
        qkT = np.empty((2, HPC * HD, s), ml_dtypes.bfloat16)
        qkT[0] = qs.transpose(1, 2, 0).reshape(HPC * HD, s)
        qkT[1] = ks_.transpose(1, 2, 0).reshape(HPC * HD, s)
        vc = np.ascontiguousarray(v[b, :, h0 * HD:(h0 + HPC) * HD]).astype(
            ml_dtypes.bfloat16
        )
        nmT = np.ascontiguousarray((~mask[b]).T)
        in_maps.append({
            "qkT": qkT,
            "v": vc,
            "nmT": nmT.astype(ml_dtypes.bfloat16),
            "nmT8": nmT.astype(ml_dtypes.float8_e4m3),
        })
    return in_maps


def assemble_out(results, s=S, d=D):
    out = np.empty((B, s, d), np.float32)
    for c in range(NCORES):
        b, g = divmod(c, NCORES // B)
        out[b, :, g * HPC * HD:(g + 1) * HPC * HD] = results[c]["out"]
    return out


def kernel(q, k, v, mask):
    nc = _get_nc()
    in_maps = make_in_maps(q, k, v, mask)
    res = run_bass_kernel_spmd(nc, in_maps, list(range(NCORES))).results
    return assemble_out(res)


# revision 72
# speedup vs baseline: 1.0015x; 1.0015x over previous
# Multi-head attention (B=2, S=2048, D=1024, H=16, head_dim=64) with bool mask,
# sharded across 8 TRN2 NeuronCores: core c -> batch c//4, heads 4*(c%4)..4*(c%4)+3.
#
# Per-core device kernel (scores computed transposed: scoresT[k, q]):
#   scoresT = K @ Q^T                 (PE bf16, lhsT = K^T strip, rhs = Q^T)
#   atp     = exp(scoresT/8)          (ACT exp scale=1/8, psum -> psum bf16)
#   at      = atp * (1-m)T            (DVE mult, psum -> SBUF bf16)
#   out[q,d] += at_chunk^T @ [V|1]    (PE bf16: lhsT = at chunk (stationary),
#                                      rhs = V'[128,65]; col 64 accumulates Z)
#   out     = psO[:, :, 0:64] / Z     (DVE reciprocal + broadcast multiply)
#
# The AV matmul uses the attention chunk as the stationary operand so the
# output lands non-transposed ([q, d] with q on partitions): free size is 65
# instead of 512 per instruction (half the PE cycles of the V-stationary
# form) and the final PE transposes disappear entirely.
#
# Host side (inside kernel()): slice per-core shards, pre-transpose Q/K per
# head ([64, S] head-dim-major, bf16), pre-transpose the inverted mask to
# bf16, reassemble the 8 per-core bf16 outputs into the full f32 output.

import sys

import numpy as np

for _p in ("/opt/trn_rl_repo",):
    if _p not in sys.path:
        sys.path.insert(0, _p)

import ml_dtypes

import concourse.bass as bass  # noqa: F401  (engine types reachable via nc)
import concourse.tile as tile
from concourse import bacc, mybir
from concourse.bass_utils import run_bass_kernel_spmd
from concourse.masks import make_identity

F32 = mybir.dt.float32
BF16 = mybir.dt.bfloat16
FP8 = mybir.dt.float8e4

S = 2048          # sequence length
HD = 64           # head dim
HPC = 4           # heads per core
NCORES = 8
B = 2
H = 16
D = H * HD

# Work-assignment schedule (per k-strip within a group), tuned against the
# cost model: which strips' exp runs as the DVE bit trick, which masks are
# folded into the QK psum accumulation on PE, which multiply on Pool.
SCHED = {
    "exp_dve": (1, 3, 5, 7, 9, 11, 13),
    "fold": (0, 1, 3, 4, 5, 7, 9, 10, 11, -2, -1),
    "pool": (6, 8),
    "lag": 7,
    "warmup": 24,
}


def build_program(s=S, reps=1):
    """Build the single-core SPMD program. Returns the compiled Bacc object.

    reps>1 emits the whole body (loads+compute+stores) that many times in one
    NEFF — used to measure device time by wall-clock differencing."""
    nc = bacc.Bacc()

    KS = s // 128            # number of k strips
    QG = 1024 if s >= 1024 else s   # q group width (ACT/DVE instruction width)
    NQG = s // QG            # q groups
    NQC = max(QG // 512, 1)  # 512-wide matmul chunks per q group (psum bank)
    QC = min(512, QG)        # matmul chunk width
    NCH = QG // 128          # 128-wide q chunks per group (AV granularity)
    CPB = 4                  # psO chunks per 2KB psum bank (zero region)
    NB = (NCH + CPB - 1) // CPB  # psO sub-tiles (1 bank each)
    LAG = min(4, KS)         # AV strips emitted this many strips behind QK

    qkT_d = nc.declare_dram_parameter("qkT", [2, HPC * HD, s], BF16, isOutput=False)
    v_d = nc.declare_dram_parameter("v", [s, HPC * HD], BF16, isOutput=False)
    nmT_d = nc.declare_dram_parameter("nmT", [s, s], BF16, isOutput=False)
    nmT8_d = nc.declare_dram_parameter("nmT8", [s, s], FP8, isOutput=False)
    out_d = nc.declare_dram_parameter("out", [s, HPC * HD], BF16, isOutput=True)

    # DRAM views with the k/q axis split into strips of 128 partitions
    nm_view = nmT_d[:].rearrange("(ks p) q -> p ks q", p=128)
    nm8_view = nmT8_d[:].rearrange("(ks p) q -> p ks q", p=128)
    v_view = v_d[:].rearrange("(ks p) (h d) -> p ks h d", p=128, h=HPC)
    out_view = out_d[:].rearrange("(sq p) c -> p sq c", p=128)

    with tile.TileContext(nc) as tc:
        with (
            tc.tile_pool(name="const", bufs=1) as const,
            tc.tile_pool(name="wq", bufs=1) as wq,
            tc.tile_pool(name="attn", bufs=20) as apool,
            tc.tile_pool(name="xsb", bufs=3) as xpool,
            tc.tile_pool(name="stat", bufs=4) as spool,
            tc.tile_pool(name="oasm", bufs=1) as opool,
            tc.tile_pool(name="psS", bufs=3, space="PSUM") as psS_pool,
            tc.tile_pool(name="psOa", bufs=1, space="PSUM") as psOa_pool,
            tc.tile_pool(name="psOb", bufs=1, space="PSUM") as psOb_pool,
        ):
            # Preload the exp table (emitted before any real exp; runs while
            # the first DMAs stream).
            warm = const.tile([128, 1], F32)
            nc.vector.memset(warm, 0.0)
            nc.scalar.activation(warm, warm, mybir.ActivationFunctionType.Exp)

            # Mask folding constants: psS += 224*nm via an fp8 DoubleRow
            # matmul (exact: the mask is 0/1 and 224 is representable in
            # e4m3 whose max is 240), then exp gets bias -28 so masked
            # entries become e^-28 ~ 7e-13 — no elementwise mask op at all.
            # DoubleRow weights: k-tile 0 = 224*I, k-tile 1 = 0 (the second
            # tile's moving data is arbitrary padding).
            identf = const.tile([128, 128], F32)
            make_identity(nc, identf)
            i448p = const.tile([128, 2, 128], FP8)
            nc.vector.memset(i448p, 0.0)
            nc.vector.tensor_scalar_mul(i448p[:, 0, :], identf, 224.0)
            bias56 = const.tile([128, 1], F32)
            nc.vector.memset(bias56, -28.0)

            # Warm the PE HAM clock gate while input DMAs run: ~3us of dummy
            # matmuls so the first real QKs run at 2.4GHz.
            zb = const.tile([128, 128], BF16)
            nc.vector.memset(zb, 0.0)
            for _ in range(SCHED.get("warmup", 24)):
                wmm = psS_pool.tile([128, QG], F32, tag="psS")
                nc.tensor.matmul(
                    wmm[:, :128], lhsT=zb[0:64, :], rhs=zb[0:64, :],
                    start=True, stop=True,
                )

            def qk_src(pair):
                return qkT_d[:, 128 * pair:128 * pair + 128, :].rearrange(
                    "t p s -> p t s"
                )

            def emit_body():
                # Q^T / K^T head pairs: [128, s] (head 2p on partitions 0-63,
                # head 2p+1 on partitions 64-127).
                qks = []
                for pair in range(HPC // 2):
                    qk = wq.tile([128, 2, s], BF16, tag=f"qkT{pair}")
                    qks.append(qk)
                # V' staging: [128, ks, h, 65] with a ones column at 64 so the
                # AV matmul's 65th output column accumulates the softmax
                # denominator Z. V lands via interleaved DMA; the ones column
                # is memset once (disjoint subtile, no dependency on the DMA).
                vps = wq.tile([128, KS, HPC, HD + 1], BF16, tag="vps")
                nm_sb = wq.tile([128, KS, s], BF16, tag="nm")
                # fp8 copy of the mask for the DoubleRow PE folds, flat with
                # a 512B tail pad so the (ignored) second k-tile window of
                # the last chunk stays in range
                nm8 = wq.tile([128, KS * s + 512], FP8, tag="nm8")
                nc.vector.memset(vps[:, :, :, HD:HD + 1], 1.0)
                nc.vector.memset(nm8[:, KS * s:], 0.0)

                # DMA choreography (s=2048): two queues only — SP (nc.sync)
                # and Pool SWDGE (nc.gpsimd) — so the ACT and DVE sequencers
                # are never blocked behind a DMA wait. Ordered by first use:
                # K strips + first Q group first, mask halves interleaved,
                # V early (AV matmuls sit in the in-order PE queue).
                QH = QG  # nm half width
                if s == 2048:
                    A, Bq = nc.sync, nc.gpsimd
                    # The model's DMA device is effectively serial, so the
                    # ordering across the queues is what matters: the
                    # first-QK inputs lead on SP (issued at t=0; the ACT
                    # queue is busy with the exp-table warmup), then mask
                    # halves at the consumption rate, with V and the second
                    # head-pair deferred to their first use.
                    A.dma_start(out=qks[0][:, 1, 0:256], in_=qk_src(0)[:, 1, 0:256])
                    A.dma_start(out=qks[0][:, 0, 0:512], in_=qk_src(0)[:, 0, 0:512])
                    A.dma_start(out=qks[0][:, 0, 512:QG], in_=qk_src(0)[:, 0, 512:QG])
                    A.dma_start(out=qks[0][:, 1, 256:1024], in_=qk_src(0)[:, 1, 256:1024])
                    Bq.dma_start(out=vps[:, :, 0, 0:HD], in_=v_view[:, :, 0])
                    A.dma_start(out=nm_sb[:, 0, 0:QH], in_=nm_view[:, 0, 0:QH])
                    Bq.dma_start(out=nm_sb[:, 1, 0:QH], in_=nm_view[:, 1, 0:QH])
                    A.dma_start(out=nm_sb[:, 2, 0:QH], in_=nm_view[:, 2, 0:QH])
                    A.dma_start(out=qks[0][:, 1, 1024:2048], in_=qk_src(0)[:, 1, 1024:2048])
                    Bq.dma_start(out=nm_sb[:, 3, 0:QH], in_=nm_view[:, 3, 0:QH])
                    for ks in range(4, KS):
                        (A if ks % 2 == 0 else Bq).dma_start(
                            out=nm_sb[:, ks, 0:QH], in_=nm_view[:, ks, 0:QH]
                        )
                        if ks == 8:
                            A.dma_start(out=qks[0][:, 0, QG:2 * QG],
                                        in_=qk_src(0)[:, 0, QG:2 * QG])
                    # second batch: q-group-1 mask halves; then the fp8 mask
                    # copy (PE folds start in group 2, late strips first),
                    # V heads 1-3 and the second head pair (needed only from
                    # groups 2/4/6 at ~33/66/100us).
                    Bq.dma_start(out=vps[:, :, 1, 0:HD], in_=v_view[:, :, 1])
                    for ks in range(KS):
                        (A if ks % 2 == 0 else Bq).dma_start(
                            out=nm_sb[:, ks, QH:2 * QH], in_=nm_view[:, ks, QH:2 * QH]
                        )
                    for ks in range(KS):
                        (A if ks % 2 == 0 else Bq).dma_start(
                            out=nm8[:, ks * s:(ks + 1) * s], in_=nm8_view[:, ks]
                        )
                    A.dma_start(out=qks[1], in_=qk_src(1))
                    Bq.dma_start(out=vps[:, :, 2, 0:HD], in_=v_view[:, :, 2])
                    Bq.dma_start(out=vps[:, :, 3, 0:HD], in_=v_view[:, :, 3])
                else:
                    A, Bq = nc.sync, nc.gpsimd
                    A.dma_start(out=qks[0], in_=qk_src(0))
                    for hh in range(HPC):
                        Bq.dma_start(out=vps[:, :, hh, 0:HD], in_=v_view[:, :, hh])
                    for pair in range(1, HPC // 2):
                        A.dma_start(out=qks[pair], in_=qk_src(pair))
                    for ks in range(KS):
                        (A if ks % 2 == 0 else Bq).dma_start(
                            out=nm_sb[:, ks, :], in_=nm_view[:, ks, :]
                        )
                    for ks in range(KS):
                        (A if ks % 2 == 0 else Bq).dma_start(
                            out=nm8[:, ks * s:(ks + 1) * s], in_=nm8_view[:, ks]
                        )

                out_asm = opool.tile([128, KS, HPC * HD], BF16)

                def _alloc_psO():
                    pools = [psOa_pool, psOb_pool]
                    tiles = []
                    for t in range(NB):
                        psO_t = pools[t].tile(
                            [128, min(CPB, NCH), 128], F32, tag=f"psO{t}"
                        )
                        tiles.append(psO_t)
                    return tiles

                def emit_carry(carry):
                    """Last two AV strips (lag-2 emission) + finalize: Z
                    reciprocal, broadcast normalize, and the output DMA once
                    the last head of a q-group completes."""
                    ch, cqg, cpsO, at_tail = carry
                    for i, (cat, cks) in enumerate(at_tail):
                        last = i == len(at_tail) - 1
                        for j in range(NCH):
                            nc.tensor.matmul(
                                cpsO[j // CPB][:, j % CPB, 0:HD + 1],
                                lhsT=cat[:, j * 128:(j + 1) * 128],
                                rhs=vps[:, cks, ch, :],
                                # start/stop once per psO sub-tile (4 chunks
                                # of 512B fill its 2KB zero region)
                                start=(cks == 0 and j % CPB == 0),
                                stop=last and (j % CPB == CPB - 1 or j == NCH - 1),
                            )
                    final = ch == HPC - 1 and cqg == NQG - 1
                    rec = spool.tile([128, NCH], F32)
                    step = min(max(NCH // 2, 1), CPB)
                    for lo in range(0, NCH, step):
                        hi = min(lo + step, NCH)
                        sq0 = cqg * NCH + lo
                        ct = cpsO[lo // CPB]
                        cl = lo % CPB
                        # per-half reciprocal over its own psO sub-tile: on
                        # the final group the first half's normalize + DMA
                        # then only depends on the first exp/mask half's AVs
                        nc.vector.reciprocal(rec[:, lo:hi], ct[:, cl:cl + hi - lo, HD])
                        nc.vector.tensor_mul(
                            out_asm[:, sq0:sq0 + hi - lo, ch * HD:(ch + 1) * HD],
                            ct[:, cl:cl + hi - lo, 0:HD],
                            rec[:, lo:hi].to_broadcast([128, hi - lo, HD]),
                        )
                        if ch == HPC - 1:
                            # the final group's DMAs both ride SP: the Pool
                            # SWDGE prep (~1.1us) would sit on the critical
                            # path at the very end of the program
                            eng = nc.gpsimd if (lo > 0 and not final) else nc.sync
                            eng.dma_start(
                                out=out_view[:, sq0:sq0 + hi - lo, :],
                                in_=out_asm[:, sq0:sq0 + hi - lo, :],
                            )

                carry = None
                groups = [(h, qg) for h in range(HPC) for qg in range(NQG)]
                NG = len(groups)

                # Work-assignment tables (balanced so each engine sits at
                # ~60-70% duty per group, with slow per-strip paths — Pool
                # masks at ~2.1us, DVE bit-trick exps — spread out so no
                # engine ever falls far enough behind to block the in-order
                # PE queue):
                #   exp:  ACT (exact) | DVE (exp2 bit trick, ~26% of strips)
                #   mask: DVE mult | POOL mult | PE (448*nm folded into QK)
                EXP_DVE_KS = {k for k in SCHED["exp_dve"] if k < max(KS - 2, 2)}
                FOLD_KS = {k % KS if k >= 0 else (KS + k) for k in SCHED["fold"]}
                POOL_KS = {k for k in SCHED["pool"] if k < KS}

                def exp_on_dve(gi, ks):
                    if gi < min(2, NG - 2):
                        return False
                    return ks in EXP_DVE_KS

                def mask_mode(gi, ks):
                    """Head groups stay on DVE: the PE fold needs the mask
                    strip ~2 strips earlier than the multiply, and the head
                    is exactly where the mask DMAs are marginal. Pool masks
                    skip group 2's first half (Pool is still issuing input
                    DMAs) and the final group's tail (a late Pool mask
                    would gate the output chain)."""
                    if gi < min(2, NG - 2):
                        return "DVE"
                    if ks in FOLD_KS:
                        return "PE"
                    if ks in POOL_KS:
                        if gi == 2 and ks < KS // 2:
                            return "DVE"
                        return "POOL"
                    return "DVE"

                for gi, (h, qg) in enumerate(groups):
                    base = 64 * (h % 2)
                    qt_r = qks[h // 2][:, 0, :]
                    kt_r = qks[h // 2][:, 1, :]
                    q0 = qg * QG
                    # AV lag: a late mask multiply (Pool backlog, in-flight
                    # nm DMA at the head) must not block the in-order PE
                    # queue right before the QK the ACT engine is waiting
                    # on. The end-of-group AV backlog overlaps the last
                    # exps (only the final strip's AVs are chain-critical).
                    lag = min(SCHED.get("lag", 4), KS)
                    last_g = gi == len(groups) - 1
                    psO = None
                    ats = {}
                    for ks in range(KS):
                        if ks - lag in ats:
                            at2 = ats.pop(ks - lag)
                            if psO is None:
                                psO = _alloc_psO()
                            for j in range(NCH):
                                nc.tensor.matmul(
                                    psO[j // CPB][:, j % CPB, 0:HD + 1],
                                    lhsT=at2[:, j * 128:(j + 1) * 128],
                                    rhs=vps[:, ks - lag, h, :],
                                    start=(ks == lag and j % CPB == 0),
                                    stop=False,
                                )
                        mmode = mask_mode(gi, ks)
                        psS = psS_pool.tile([128, QG], F32, tag="psS")
                        for qc in range(NQC):
                            qsl = slice(q0 + qc * QC, q0 + (qc + 1) * QC)
                            nc.tensor.matmul(
                                psS[:, qc * QC:(qc + 1) * QC],
                                lhsT=kt_r[base:base + HD, ks * 128:(ks + 1) * 128],
                                rhs=qt_r[base:base + HD, qsl],
                                start=True,
                                stop=(mmode != "PE"),
                            )
                            if mmode == "PE":
                                # fold the mask into the psum accumulation
                                # (psS += 224*nm; exp then gets bias -28) as an
                                # fp8 DoubleRow matmul: exact (mask is 0/1,
                                # 224 is representable) at half the cycles;
                                # k-tile 1 has zero weights so its moving
                                # window is don't-care.
                                off = ks * s + qsl.start
                                rhs8 = nm8[:, off:off + 2 * QC].rearrange(
                                    "p (two f) -> p two f", two=2
                                )
                                nc.tensor.matmul(
                                    psS[:, qc * QC:(qc + 1) * QC],
                                    lhsT=i448p,
                                    rhs=rhs8,
                                    start=False,
                                    stop=True,
                                    perf_mode=mybir.MatmulPerfMode.DoubleRow,
                                )

                        at = apool.tile([128, QG], BF16, tag="at")
                        folded = mmode == "PE"
                        if exp_on_dve(gi, ks):
                            # exp2 bit trick on DVE: y = trunc(x*0.125*
                            # log2(e)*128 + (127*128 - 5.5)) as int16 IS the
                            # bf16 pattern of ~exp(x/8) (softmax
                            # normalization absorbs the approximation's
                            # constant factor; the -5.5 centers its mean so
                            # mixing with exact-exp strips stays unbiased).
                            # With the PE mask fold, the -448 lands in the
                            # exponent field: masked entries become ~2^-69.
                            c1 = 0.125 * 1.4426950408889634 * 128.0
                            c2 = 16250.5 - (224.0 * c1 if folded else 0.0)
                            nc.vector.tensor_scalar(
                                at[:].bitcast(mybir.dt.int16), psS,
                                c1, c2,
                                mybir.AluOpType.mult,
                                mybir.AluOpType.add,
                            )
                        else:
                            # last strip of every group: exp (+mask) in
                            # halves so the carry AV chunks 0-3 start half
                            # an exp earlier (subtile deps)
                            nsp = 2 if (ks == KS - 1 and QG >= 1024) else 1
                            for sp in range(nsp):
                                sl = slice(sp * QG // nsp, (sp + 1) * QG // nsp)
                                nc.scalar.activation(
                                    at[:, sl], psS[:, sl],
                                    mybir.ActivationFunctionType.Exp,
                                    scale=0.125,
                                    bias=bias56[:] if folded else 0.0,
                                )
                                if mmode == "DVE":
                                    nc.vector.tensor_mul(
                                        at[:, sl], at[:, sl],
                                        nm_sb[:, ks, q0 + sl.start:q0 + sl.stop],
                                    )
                        if not folded and (mmode == "POOL" or exp_on_dve(gi, ks)):
                            eng = nc.gpsimd if mmode == "POOL" else nc.vector
                            eng.tensor_mul(at, at, nm_sb[:, ks, q0:q0 + QG])
                        ats[ks] = at
                        # carry (norms on DVE) emitted after strip 1's ops:
                        # at a group boundary the new group's strip-1 DVE
                        # affine must not queue behind the old group's
                        # normalize chain, or the psS rotation stalls ACT
                        if ks == min(1, KS - 1) and carry is not None:
                            emit_carry(carry)
                            carry = None
                    if psO is None:
                        psO = _alloc_psO()
                    tail = sorted(ats.items())
                    carry = (h, qg, psO, [(a, k) for k, a in tail])
                emit_carry(carry)

            for _ in range(reps):
                emit_body()
    nc.compile()
    return nc


_CACHE = {}


def _get_nc():
    if "nc" not in _CACHE:
        _CACHE["nc"] = build_program()
    return _CACHE["nc"]


def make_in_maps(q, k, v, mask, s=S):
    """Shard full inputs into 8 per-core input maps (host-side layout prep)."""
    q = np.asarray(q, dtype=np.float32)
    k = np.asarray(k, dtype=np.float32)
    v = np.asarray(v, dtype=np.float32)
    mask = np.asarray(mask)
    nh = q.shape[-1] // HD
    in_maps = []
    for c in range(NCORES):
        b, g = divmod(c, NCORES // B)
        h0 = HPC * g
        qs = q[b].reshape(s, nh, HD)[:, h0:h0 + HPC, :]      # [s, HPC, 64]
        ks_ = k[b].reshape(s, nh, HD)[:, h0:h0 + HPC, :]
        qkT = np.empty((2, HPC * HD, s), ml_dtypes.bfloat16)
        qkT[0] = qs.transpose(1, 2, 0).reshape(HPC * HD, s)
        qkT[1] = ks_.transpose(1, 2, 0).reshape(HPC * HD, s)
        vc = np.ascontiguousarray(v[b, :, h0 * HD:(h0 + HPC) * HD]).astype(
            ml_dtypes.bfloat16
        )
        nmT = np.ascontiguousarray((~mask[b]).T)
        in_maps.append({
            "qkT": qkT,
            "v": vc,
            "nmT": nmT.astype(ml_dtypes.bfloat16),
            "nmT8": nmT.astype(ml_dtypes.float8_e4m3),
        })
    return in_maps


def assemble_out(results, s=S, d=D):
    out = np.empty((B, s, d), np.float32)
    for c in range(NCORES):
        b, g = divmod(c, NCORES // B)
        out[b, :, g * HPC * HD:(g + 1) * HPC * HD] = results[c]["out"]
    return out


def kernel(q, k, v, mask):
    nc = _get_nc()
    in_maps = make_in_maps(q, k, v, mask)
    res = run_bass_kernel_spmd(nc, in_maps, list(range(NCORES))).results
    return assemble_out(res)


# revision 73
# speedup vs baseline: 1.0133x; 1.0118x over previous
# Multi-head attention (B=2, S=2048, D=1024, H=16, head_dim=64) with bool mask,
# sharded across 8 TRN2 NeuronCores: core c -> batch c//4, heads 4*(c%4)..4*(c%4)+3.
#
# Per-core device kernel (scores computed transposed: scoresT[k, q]):
#   scoresT = K @ Q^T                 (PE bf16, lhsT = K^T strip, rhs = Q^T)
#   atp     = exp(scoresT/8)          (ACT exp scale=1/8, psum -> psum bf16)
#   at      = atp * (1-m)T            (DVE mult, psum -> SBUF bf16)
#   out[q,d] += at_chunk^T @ [V|1]    (PE bf16: lhsT = at chunk (stationary),
#                                      rhs = V'[128,65]; col 64 accumulates Z)
#   out     = psO[:, :, 0:64] / Z     (DVE reciprocal + broadcast multiply)
#
# The AV matmul uses the attention chunk as the stationary operand so the
# output lands non-transposed ([q, d] with q on partitions): free size is 65
# instead of 512 per instruction (half the PE cycles of the V-stationary
# form) and the final PE transposes disappear entirely.
#
# Host side (inside kernel()): slice per-core shards, pre-transpose Q/K per
# head ([64, S] head-dim-major, bf16), pre-transpose the inverted mask to
# bf16, reassemble the 8 per-core bf16 outputs into the full f32 output.

import sys

import numpy as np

for _p in ("/opt/trn_rl_repo",):
    if _p not in sys.path:
        sys.path.insert(0, _p)

import ml_dtypes

import concourse.bass as bass  # noqa: F401  (engine types reachable via nc)
import concourse.tile as tile
from concourse import bacc, mybir
from concourse.bass_utils import run_bass_kernel_spmd
from concourse.masks import make_identity

F32 = mybir.dt.float32
BF16 = mybir.dt.bfloat16
FP8 = mybir.dt.float8e4

S = 2048          # sequence length
HD = 64           # head dim
HPC = 4           # heads per core
NCORES = 8
B = 2
H = 16
D = H * HD

# Work-assignment schedule (per k-strip within a group), tuned against the
# cost model: which strips' exp runs as the DVE bit trick, which masks are
# folded into the QK psum accumulation on PE, which multiply on Pool.
SCHED = {
    "exp_dve": (1, 3, 5, 7, 9, 11, 13),
    "fold": (0, 1, 3, 4, 5, 7, 9, 10, 11, -2, -1),
    "pool": (6, 8),
    "lag": 7,
    "warmup": 24,
}


def build_program(s=S, reps=1):
    """Build the single-core SPMD program. Returns the compiled Bacc object.

    reps>1 emits the whole body (loads+compute+stores) that many times in one
    NEFF — used to measure device time by wall-clock differencing."""
    nc = bacc.Bacc()

    KS = s // 128            # number of k strips
    QG = 1024 if s >= 1024 else s   # q group width (ACT/DVE instruction width)
    NQG = s // QG            # q groups
    NQC = max(QG // 512, 1)  # 512-wide matmul chunks per q group (psum bank)
    QC = min(512, QG)        # matmul chunk width
    NCH = QG // 128          # 128-wide q chunks per group (AV granularity)
    CPB = 4                  # psO chunks per 2KB psum bank (zero region)
    NB = (NCH + CPB - 1) // CPB  # psO sub-tiles (1 bank each)
    LAG = min(4, KS)         # AV strips emitted this many strips behind QK

    qkT_d = nc.declare_dram_parameter("qkT", [2, HPC * HD, s], BF16, isOutput=False)
    v_d = nc.declare_dram_parameter("v", [s, HPC * HD], BF16, isOutput=False)
    nmT_d = nc.declare_dram_parameter("nmT", [s, s], BF16, isOutput=False)
    nmT8_d = nc.declare_dram_parameter("nmT8", [s, s], FP8, isOutput=False)
    out_d = nc.declare_dram_parameter("out", [s, HPC * HD], BF16, isOutput=True)

    # DRAM views with the k/q axis split into strips of 128 partitions
    nm_view = nmT_d[:].rearrange("(ks p) q -> p ks q", p=128)
    nm8_view = nmT8_d[:].rearrange("(ks p) q -> p ks q", p=128)
    v_view = v_d[:].rearrange("(ks p) (h d) -> p ks h d", p=128, h=HPC)
    out_view = out_d[:].rearrange("(sq p) c -> p sq c", p=128)

    with tile.TileContext(nc) as tc:
        with (
            tc.tile_pool(name="const", bufs=1) as const,
            tc.tile_pool(name="wq", bufs=1) as wq,
            tc.tile_pool(name="attn", bufs=20) as apool,
            tc.tile_pool(name="xsb", bufs=3) as xpool,
            tc.tile_pool(name="stat", bufs=4) as spool,
            tc.tile_pool(name="oasm", bufs=1) as opool,
            tc.tile_pool(name="psS", bufs=3, space="PSUM") as psS_pool,
            tc.tile_pool(name="psOa", bufs=1, space="PSUM") as psOa_pool,
            tc.tile_pool(name="psOb", bufs=1, space="PSUM") as psOb_pool,
        ):
            # Preload the exp table (emitted before any real exp; runs while
            # the first DMAs stream).
            warm = const.tile([128, 1], F32)
            nc.vector.memset(warm, 0.0)
            nc.scalar.activation(warm, warm, mybir.ActivationFunctionType.Exp)

            # Mask folding constants: psS += 224*nm via an fp8 DoubleRow
            # matmul (exact: the mask is 0/1 and 224 is representable in
            # e4m3 whose max is 240), then exp gets bias -28 so masked
            # entries become e^-28 ~ 7e-13 — no elementwise mask op at all.
            # DoubleRow weights: k-tile 0 = 224*I, k-tile 1 = 0 (the second
            # tile's moving data is arbitrary padding).
            identf = const.tile([128, 128], F32)
            make_identity(nc, identf)
            i448p = const.tile([128, 2, 128], FP8)
            nc.vector.memset(i448p, 0.0)
            nc.vector.tensor_scalar_mul(i448p[:, 0, :], identf, 224.0)
            bias56 = const.tile([128, 1], F32)
            nc.vector.memset(bias56, -28.0)

            # Warm the PE HAM clock gate while input DMAs run: ~3us of dummy
            # matmuls so the first real QKs run at 2.4GHz.
            zb = const.tile([128, 128], BF16)
            nc.vector.memset(zb, 0.0)
            for _ in range(SCHED.get("warmup", 24)):
                wmm = psS_pool.tile([128, QG], F32, tag="psS")
                nc.tensor.matmul(
                    wmm[:, :128], lhsT=zb[0:64, :], rhs=zb[0:64, :],
                    start=True, stop=True,
                )

            def qk_src(pair):
                return qkT_d[:, 128 * pair:128 * pair + 128, :].rearrange(
                    "t p s -> p t s"
                )

            def emit_body():
                # Q^T / K^T head pairs: [128, s] (head 2p on partitions 0-63,
                # head 2p+1 on partitions 64-127).
                qks = []
                for pair in range(HPC // 2):
                    qk = wq.tile([128, 2, s], BF16, tag=f"qkT{pair}")
                    qks.append(qk)
                # V' staging: [128, ks, h, 65] with a ones column at 64 so the
                # AV matmul's 65th output column accumulates the softmax
                # denominator Z. V lands via interleaved DMA; the ones column
                # is memset once (disjoint subtile, no dependency on the DMA).
                vps = wq.tile([128, KS, HPC, HD + 1], BF16, tag="vps")
                nm_sb = wq.tile([128, KS, s], BF16, tag="nm")
                # fp8 copy of the mask for the DoubleRow PE folds, flat with
                # a 512B tail pad so the (ignored) second k-tile window of
                # the last chunk stays in range
                nm8 = wq.tile([128, KS * s + 512], FP8, tag="nm8")
                nc.vector.memset(vps[:, :, :, HD:HD + 1], 1.0)
                nc.vector.memset(nm8[:, KS * s:], 0.0)

                # DMA choreography (s=2048): two queues only — SP (nc.sync)
                # and Pool SWDGE (nc.gpsimd) — so the ACT and DVE sequencers
                # are never blocked behind a DMA wait. Ordered by first use:
                # K strips + first Q group first, mask halves interleaved,
                # V early (AV matmuls sit in the in-order PE queue).
                QH = QG  # nm half width
                if s == 2048:
                    A, Bq = nc.sync, nc.gpsimd
                    # The model's DMA device is effectively serial, so the
                    # ordering across the queues is what matters: the
                    # first-QK inputs lead on SP (issued at t=0; the ACT
                    # queue is busy with the exp-table warmup), then mask
                    # halves at the consumption rate, with V and the second
                    # head-pair deferred to their first use.
                    A.dma_start(out=qks[0][:, 1, 0:512], in_=qk_src(0)[:, 1, 0:512])
                    A.dma_start(out=qks[0][:, 0, 0:QG], in_=qk_src(0)[:, 0, 0:QG])
                    A.dma_start(out=qks[0][:, 1, 512:1024], in_=qk_src(0)[:, 1, 512:1024])
                    Bq.dma_start(out=vps[:, :, 0, 0:HD], in_=v_view[:, :, 0])
                    A.dma_start(out=nm_sb[:, 0, 0:QH], in_=nm_view[:, 0, 0:QH])
                    Bq.dma_start(out=nm_sb[:, 1, 0:QH], in_=nm_view[:, 1, 0:QH])
                    A.dma_start(out=nm_sb[:, 2, 0:QH], in_=nm_view[:, 2, 0:QH])
                    A.dma_start(out=qks[0][:, 1, 1024:2048], in_=qk_src(0)[:, 1, 1024:2048])
                    Bq.dma_start(out=nm_sb[:, 3, 0:QH], in_=nm_view[:, 3, 0:QH])
                    for ks in range(4, KS):
                        (A if ks % 2 == 0 else Bq).dma_start(
                            out=nm_sb[:, ks, 0:QH], in_=nm_view[:, ks, 0:QH]
                        )
                        if ks == 8:
                            A.dma_start(out=qks[0][:, 0, QG:2 * QG],
                                        in_=qk_src(0)[:, 0, QG:2 * QG])
                    # second batch: q-group-1 mask halves; then the fp8 mask
                    # copy (PE folds start in group 2, late strips first),
                    # V heads 1-3 and the second head pair (needed only from
                    # groups 2/4/6 at ~33/66/100us).
                    Bq.dma_start(out=vps[:, :, 1, 0:HD], in_=v_view[:, :, 1])
                    for ks in range(KS):
                        (A if ks % 2 == 0 else Bq).dma_start(
                            out=nm_sb[:, ks, QH:2 * QH], in_=nm_view[:, ks, QH:2 * QH]
                        )
                    for ks in range(KS):
                        (A if ks % 2 == 0 else Bq).dma_start(
                            out=nm8[:, ks * s:(ks + 1) * s], in_=nm8_view[:, ks]
                        )
                    A.dma_start(out=qks[1], in_=qk_src(1))
                    Bq.dma_start(out=vps[:, :, 2, 0:HD], in_=v_view[:, :, 2])
                    Bq.dma_start(out=vps[:, :, 3, 0:HD], in_=v_view[:, :, 3])
                else:
                    A, Bq = nc.sync, nc.gpsimd
                    A.dma_start(out=qks[0], in_=qk_src(0))
                    for hh in range(HPC):
                        Bq.dma_start(out=vps[:, :, hh, 0:HD], in_=v_view[:, :, hh])
                    for pair in range(1, HPC // 2):
                        A.dma_start(out=qks[pair], in_=qk_src(pair))
                    for ks in range(KS):
                        (A if ks % 2 == 0 else Bq).dma_start(
                            out=nm_sb[:, ks, :], in_=nm_view[:, ks, :]
                        )
                    for ks in range(KS):
                        (A if ks % 2 == 0 else Bq).dma_start(
                            out=nm8[:, ks * s:(ks + 1) * s], in_=nm8_view[:, ks]
                        )

                out_asm = opool.tile([128, KS, HPC * HD], BF16)

                def _alloc_psO():
                    pools = [psOa_pool, psOb_pool]
                    tiles = []
                    for t in range(NB):
                        psO_t = pools[t].tile(
                            [128, min(CPB, NCH), 128], F32, tag=f"psO{t}"
                        )
                        tiles.append(psO_t)
                    return tiles

                def emit_carry(carry):
                    """Last two AV strips (lag-2 emission) + finalize: Z
                    reciprocal, broadcast normalize, and the output DMA once
                    the last head of a q-group completes."""
                    ch, cqg, cpsO, at_tail = carry
                    for i, (cat, cks) in enumerate(at_tail):
                        last = i == len(at_tail) - 1
                        for j in range(NCH):
                            nc.tensor.matmul(
                                cpsO[j // CPB][:, j % CPB, 0:HD + 1],
                                lhsT=cat[:, j * 128:(j + 1) * 128],
                                rhs=vps[:, cks, ch, :],
                                # start/stop once per psO sub-tile (4 chunks
                                # of 512B fill its 2KB zero region)
                                start=(cks == 0 and j % CPB == 0),
                                stop=last and (j % CPB == CPB - 1 or j == NCH - 1),
                            )
                    final = ch == HPC - 1 and cqg == NQG - 1
                    rec = spool.tile([128, NCH], F32)
                    step = min(max(NCH // 2, 1), CPB)
                    for lo in range(0, NCH, step):
                        hi = min(lo + step, NCH)
                        sq0 = cqg * NCH + lo
                        ct = cpsO[lo // CPB]
                        cl = lo % CPB
                        # per-half reciprocal over its own psO sub-tile: on
                        # the final group the first half's normalize + DMA
                        # then only depends on the first exp/mask half's AVs
                        nc.vector.reciprocal(rec[:, lo:hi], ct[:, cl:cl + hi - lo, HD])
                        nc.vector.tensor_mul(
                            out_asm[:, sq0:sq0 + hi - lo, ch * HD:(ch + 1) * HD],
                            ct[:, cl:cl + hi - lo, 0:HD],
                            rec[:, lo:hi].to_broadcast([128, hi - lo, HD]),
                        )
                        if ch == HPC - 1:
                            # the final group's DMAs both ride SP: the Pool
                            # SWDGE prep (~1.1us) would sit on the critical
                            # path at the very end of the program
                            eng = nc.gpsimd if (lo > 0 and not final) else nc.sync
                            eng.dma_start(
                                out=out_view[:, sq0:sq0 + hi - lo, :],
                                in_=out_asm[:, sq0:sq0 + hi - lo, :],
                            )

                carry = None
                groups = [(h, qg) for h in range(HPC) for qg in range(NQG)]
                NG = len(groups)

                # Work-assignment tables (balanced so each engine sits at
                # ~60-70% duty per group, with slow per-strip paths — Pool
                # masks at ~2.1us, DVE bit-trick exps — spread out so no
                # engine ever falls far enough behind to block the in-order
                # PE queue):
                #   exp:  ACT (exact) | DVE (exp2 bit trick, ~26% of strips)
                #   mask: DVE mult | POOL mult | PE (448*nm folded into QK)
                EXP_DVE_KS = {k for k in SCHED["exp_dve"] if k < max(KS - 2, 2)}
                FOLD_KS = {k % KS if k >= 0 else (KS + k) for k in SCHED["fold"]}
                POOL_KS = {k for k in SCHED["pool"] if k < KS}

                def exp_on_dve(gi, ks):
                    if gi < min(2, NG - 2):
                        return False
                    return ks in EXP_DVE_KS

                def mask_mode(gi, ks):
                    """Head groups stay on DVE: the PE fold needs the mask
                    strip ~2 strips earlier than the multiply, and the head
                    is exactly where the mask DMAs are marginal. Pool masks
                    skip group 2's first half (Pool is still issuing input
                    DMAs) and the final group's tail (a late Pool mask
                    would gate the output chain)."""
                    if gi < min(2, NG - 2):
                        return "DVE"
                    if ks in FOLD_KS:
                        return "PE"
                    if ks in POOL_KS:
                        if gi == 2 and ks < KS // 2:
                            return "DVE"
                        return "POOL"
                    return "DVE"

                for gi, (h, qg) in enumerate(groups):
                    base = 64 * (h % 2)
                    qt_r = qks[h // 2][:, 0, :]
                    kt_r = qks[h // 2][:, 1, :]
                    q0 = qg * QG
                    # AV lag: a late mask multiply (Pool backlog, in-flight
                    # nm DMA at the head) must not block the in-order PE
                    # queue right before the QK the ACT engine is waiting
                    # on. The end-of-group AV backlog overlaps the last
                    # exps (only the final strip's AVs are chain-critical).
                    lag = min(SCHED.get("lag", 4), KS)
                    last_g = gi == len(groups) - 1
                    psO = None
                    ats = {}
                    for ks in range(KS):
                        if ks - lag in ats:
                            at2 = ats.pop(ks - lag)
                            if psO is None:
                                psO = _alloc_psO()
                            for j in range(NCH):
                                nc.tensor.matmul(
                                    psO[j // CPB][:, j % CPB, 0:HD + 1],
                                    lhsT=at2[:, j * 128:(j + 1) * 128],
                                    rhs=vps[:, ks - lag, h, :],
                                    start=(ks == lag and j % CPB == 0),
                                    stop=False,
                                )
                        mmode = mask_mode(gi, ks)
                        psS = psS_pool.tile([128, QG], F32, tag="psS")
                        for qc in range(NQC):
                            qsl = slice(q0 + qc * QC, q0 + (qc + 1) * QC)
                            nc.tensor.matmul(
                                psS[:, qc * QC:(qc + 1) * QC],
                                lhsT=kt_r[base:base + HD, ks * 128:(ks + 1) * 128],
                                rhs=qt_r[base:base + HD, qsl],
                                start=True,
                                stop=(mmode != "PE"),
                            )
                            if mmode == "PE":
                                # fold the mask into the psum accumulation
                                # (psS += 224*nm; exp then gets bias -28) as an
                                # fp8 DoubleRow matmul: exact (mask is 0/1,
                                # 224 is representable) at half the cycles;
                                # k-tile 1 has zero weights so its moving
                                # window is don't-care.
                                off = ks * s + qsl.start
                                rhs8 = nm8[:, off:off + 2 * QC].rearrange(
                                    "p (two f) -> p two f", two=2
                                )
                                nc.tensor.matmul(
                                    psS[:, qc * QC:(qc + 1) * QC],
                                    lhsT=i448p,
                                    rhs=rhs8,
                                    start=False,
                                    stop=True,
                                    perf_mode=mybir.MatmulPerfMode.DoubleRow,
                                )

                        at = apool.tile([128, QG], BF16, tag="at")
                        folded = mmode == "PE"
                        if exp_on_dve(gi, ks):
                            # exp2 bit trick on DVE: y = trunc(x*0.125*
                            # log2(e)*128 + (127*128 - 5.5)) as int16 IS the
                            # bf16 pattern of ~exp(x/8) (softmax
                            # normalization absorbs the approximation's
                            # constant factor; the -5.5 centers its mean so
                            # mixing with exact-exp strips stays unbiased).
                            # With the PE mask fold, the -448 lands in the
                            # exponent field: masked entries become ~2^-69.
                            c1 = 0.125 * 1.4426950408889634 * 128.0
                            c2 = 16250.5 - (224.0 * c1 if folded else 0.0)
                            nc.vector.tensor_scalar(
                                at[:].bitcast(mybir.dt.int16), psS,
                                c1, c2,
                                mybir.AluOpType.mult,
                                mybir.AluOpType.add,
                            )
                        else:
                            # last strip of every group: exp (+mask) in
                            # halves so the carry AV chunks 0-3 start half
                            # an exp earlier (subtile deps)
                            nsp = 2 if (ks == KS - 1 and QG >= 1024) else 1
                            for sp in range(nsp):
                                sl = slice(sp * QG // nsp, (sp + 1) * QG // nsp)
                                nc.scalar.activation(
                                    at[:, sl], psS[:, sl],
                                    mybir.ActivationFunctionType.Exp,
                                    scale=0.125,
                                    bias=bias56[:] if folded else 0.0,
                                )
                                if mmode == "DVE":
                                    nc.vector.tensor_mul(
                                        at[:, sl], at[:, sl],
                                        nm_sb[:, ks, q0 + sl.start:q0 + sl.stop],
                                    )
                        if not folded and (mmode == "POOL" or exp_on_dve(gi, ks)):
                            eng = nc.gpsimd if mmode == "POOL" else nc.vector
                            eng.tensor_mul(at, at, nm_sb[:, ks, q0:q0 + QG])
                        ats[ks] = at
                        # carry (norms on DVE) emitted after strip 1's ops:
                        # at a group boundary the new group's strip-1 DVE
                        # affine must not queue behind the old group's
                        # normalize chain, or the psS rotation stalls ACT
                        if ks == min(1, KS - 1) and carry is not None:
                            emit_carry(carry)
                            carry = None
                    if psO is None:
                        psO = _alloc_psO()
                    tail = sorted(ats.items())
                    carry = (h, qg, psO, [(a, k) for k, a in tail])
                emit_carry(carry)

            for _ in range(reps):
                emit_body()
    nc.compile()
    return nc


_CACHE = {}


def _get_nc():
    if "nc" not in _CACHE:
        _CACHE["nc"] = build_program()
    return _CACHE["nc"]


def make_in_maps(q, k, v, mask, s=S):
    """Shard full inputs into 8 per-core input maps (host-side layout prep)."""
    q = np.asarray(q, dtype=np.float32)
    k = np.asarray(k, dtype=np.float32)
    v = np.asarray(v, dtype=np.float32)
    mask = np.asarray(mask)
    nh = q.shape[-1] // HD
    in_maps = []
    for c in range(NCORES):
        b, g = divmod(c, NCORES // B)
        h0 = HPC * g
        qs = q[b].reshape(s, nh, HD)[:, h0:h0 + HPC, :]      # [s, HPC, 64]
        ks_ = k[b].reshape(s, nh, HD)[:, h0:h0 + HPC, :]
        qkT = np.empty((2, HPC * HD, s), ml_dtypes.bfloat16)
        qkT[0] = qs.transpose(1, 2, 0).reshape(HPC * HD, s)
        qkT[1] = ks_.transpose(1, 2, 0).reshape(HPC * HD, s)
        vc = np.ascontiguousarray(v[b, :, h0 * HD:(h0 + HPC) * HD]).astype(
            ml_dtypes.bfloat16
        )
        nmT = np.ascontiguousarray((~mask[b]).T)
        in_maps.append({
            "qkT": qkT,
            "v": vc,
            "nmT": nmT.astype(ml_dtypes.bfloat16),
            "nmT8": nmT.astype(ml_dtypes.float8_e4m3),
        })
    return in_maps


def assemble_out(results, s=S, d=D):
    out = np.empty((B, s, d), np.float32)
    for c in range(NCORES):
        b, g = divmod(c, NCORES // B)
        out[b, :, g * HPC * HD:(g + 1) * HPC * HD] = results[c]["out"]
    return out


def kernel(q, k, v, mask):
    nc = _get_nc()
    in_maps = make_in_maps(q, k, v, mask)
    res = run_bass_kernel_spmd(nc, in_maps, list(range(NCORES))).results
    return assemble_out(res)
